# revision 41
# baseline (speedup 1.0000x reference)
"""TopK sparse autoencoder forward pass on 8 Trainium2 NeuronCores.

Math (reference):
    preact = (x - b_dec) @ W_enc.T + b_enc          # [B, F]
    top32 = exact per-row top-32 of relu(preact)
    x_hat = scatter(top32) @ W_dec.T + b_dec        # [B, D]

Strategy: data-parallel over batch rows (1024 rows/core, no collectives).
Per core (DESIGN="top8", ENC_MODE="fp16fp8" - the measured-best config):
  encode: preact computed at ~2^-17 precision as fp16 main + fp8e4
          DoubleRow corrections in ONE PSUM accumulation group per
          [128 x 512] block. Host pre-scales the fp16 operands by 2^8/2^7
          and the fp8 splits by 2^11/2^4 and 2^0/2^15 so every term lands
          at scale 2^15 (no evacuation math; 2^-15 folds into the decode
          coefficients). W streamed block-by-block, x resident in SBUF.
  topk:   per block, DVE max8/max_index capture the block's top-8
          values+indices straight from PSUM (no preact spill, no candidate
          gather). 256 candidates/row provably cover the true top-32
          unless one 512-block holds >8 winners (P ~ 3.6e-5 per row).
          Tail per 128-row tile: 4 max/max_index/match_replace rounds over
          the 256 candidates, then a one-hot uint16 lookup maps positions
          to feature indices.
  decode: gather the 32 selected W_dec.T rows per row (fp16, one [128,1]
          indirect DMA per coefficient - multi-offset gathers are broken
          on this HW) and accumulate sum_c diag(vals[:,c]) @ G_c on the PE.
  overlap: batch is processed in HALVES groups; the tails of group i run
          under the encode of group i+1 (W re-streamed per group, DMA has
          headroom), so only the last group's tails are exposed.

Selection exactness: preact error must stay below ~1e-4 (top-32 boundary
gaps ~0.025; a swapped feature decodes to an orthogonal direction). The
fp16+fp8 split gives ~2e-5; 1-pass bf16/fp16/f32r all fail this bar.

Host side reshapes/transposes inputs, builds the fp16/fp8 splits, and
converts W_dec.T to fp16.
"""
import sys
sys.path.insert(0, '/opt/trn_rl_repo')

import numpy as np

B, D, F, K = 8192, 768, 16384, 32
N_CORES = 8
BC = B // N_CORES          # rows per core (1024)
NBT = BC // 128            # b-tiles per core (8)
NFB = F // 512             # f-blocks (32)
KD = D // 128              # contraction chunks (6)
CH = 64                    # topk chunk width
C = F // CH                # chunks per row (256)
NCH = K // 8               # rounds of 8 (4)

ENC_MODE = "fp16fp8"       # "fp32" | "f32r" (broken) | "bf16x3" | "fp16fp8"
HALVES = 2                 # split batch to overlap encode with topk/decode
BATCH_GATHER = False       # multi-offset indirect DMA: BROKEN on HW (garbage)
DESIGN = "top8"            # "spill" (DRAM preact spill) | "top8" (block top-8)
FIX_SYNC = True            # walrus needs split sync waits; CoreSim chokes on them
DEBUG_TOP8 = False         # dump bt=0 intermediates as extra outputs
FP8_PLAIN = False          # corrections as 12 plain fp8 matmuls (FWL) vs 6 DR
DVE_DECODE = False         # decode via DVE mult+reduce: slower on HW than PE
LDW_OPT = False            # flip walrus --enable-ldw-opt (hides weight loads)

_cache = {}


def _fix_sync_waits(nc, maxw=1):
    """This container's walrus rejects >1 sync wait per instruction; split
    excess waits onto same-engine NoOps inserted just before."""
    import bass_rust
    import concourse.mybir as mybir
    ctr = 0
    for f in nc.m.functions:
        for bb in f.blocks:
            out, changed = [], False
            for inst in bb.instructions:
                si = inst.sync_info
                waits = list(si.on_wait) if si is not None else []
                if len(waits) > maxw:
                    changed = True
                    head, keep = waits[:-maxw], waits[-maxw:]
                    for i in range(0, len(head), maxw):
                        ctr += 1
                        nop = mybir.InstNoOp(
                            name=f"syncfix-nop-{id(nc)}-{ctr}", ins=[], outs=[])
                        nop.engine = inst.engine
                        nop.sync_info = bass_rust.SyncInfo(
                            on_wait=head[i:i + maxw], on_update=[])
                        out.append(nop)
                    si.on_wait = keep
                out.append(inst)
            if changed:
                bb.instructions = out


def _build_top8(has_benc: bool, has_bdec: bool, repeat: int = 1):
    """Spill-free design: per 512-wide f-block keep the block's top-8
    values+indices (DVE max8); per row that leaves 256 candidates covering
    the true top-32 unless one block holds >8 of them (P ~ 3.6e-5 per row;
    a miss costs one tail coefficient of one row - far inside tolerance)."""
    import concourse.bass as bass
    import concourse.mybir as mybir
    import concourse.tile as tile
    dt = mybir.dt
    Alu = mybir.AluOpType

    nc = bass.Bass("TRN2", target_bir_lowering=False, debug=False,
                   num_devices=N_CORES)

    if ENC_MODE == "fp16fp8":
        # main term xh16*wh16 (fp16) + corrections (xl*wh + xh*wl)*2^15 via
        # one fp8e4 DoubleRow accumulation chain (slot0: xl8*wh8, slot1:
        # xh8*wl8); evacuation recombines ps_main + 2^-15*ps_corr.
        xTh_d = nc.dram_tensor("xTh", [D, BC], dt.float16,
                               kind="ExternalInput")
        xc8_d = nc.dram_tensor("xc8", [2 * D, BC], dt.float8e4,
                               kind="ExternalInput")
        wencTh_d = nc.dram_tensor("wencTh", [D, F], dt.float16,
                                  kind="ExternalInput")
        wc8_d = nc.dram_tensor("wc8", [2 * D, F], dt.float8e4,
                               kind="ExternalInput")
    else:
        assert ENC_MODE == "bf16x3"
        xTh_d = nc.dram_tensor("xTh", [D, BC], dt.bfloat16,
                               kind="ExternalInput")
        xTl_d = nc.dram_tensor("xTl", [D, BC], dt.bfloat16,
                               kind="ExternalInput")
        wencTh_d = nc.dram_tensor("wencTh", [D, F], dt.bfloat16,
                                  kind="ExternalInput")
        wencTl_d = nc.dram_tensor("wencTl", [D, F], dt.bfloat16,
                                  kind="ExternalInput")
    wdecT_d = nc.dram_tensor("wdecT16", [F, D], dt.float16, kind="ExternalInput")
    beff_d = nc.dram_tensor("beff", [1, F], dt.float32, kind="ExternalInput")
    bdec_d = nc.dram_tensor("bdec", [1, D], dt.float32, kind="ExternalInput")
    out_d = nc.dram_tensor("xhat", [BC, D], dt.float32, kind="ExternalOutput")

    NC8 = NFB * 8              # candidates per row (256)
    if DEBUG_TOP8:
        dbg_v = nc.dram_tensor("dbg_v", [128, NC8], dt.float32,
                               kind="ExternalOutput")
        dbg_i = nc.dram_tensor("dbg_i", [128, NC8], dt.uint16,
                               kind="ExternalOutput")
        dbg_gidx = nc.dram_tensor("dbg_gidx", [128, NC8], dt.uint16,
                                  kind="ExternalOutput")
        dbg_tpos = nc.dram_tensor("dbg_tpos", [128, K], dt.uint16,
                                  kind="ExternalOutput")
        dbg_vals = nc.dram_tensor("dbg_vals", [128, K], dt.float32,
                                  kind="ExternalOutput")
        dbg_idx = nc.dram_tensor("dbg_idx", [128, K], dt.uint32,
                                 kind="ExternalOutput")

    def body(tc, pools):
        sb, sb1, stp, psA, psB, psC = pools

        if ENC_MODE == "fp16fp8":
            xTh = sb1.tile([128, KD, BC], dt.float16)
            nc.sync.dma_start(
                xTh[:], xTh_d.ap().rearrange("(po pi) b -> pi po b", pi=128))
            xc8 = sb1.tile([128, KD, 2, BC], dt.float8e4)
            nc.sync.dma_start(
                xc8[:],
                xc8_d.ap().rearrange("(kp i pi) b -> pi kp i b", pi=128, i=2))
        else:
            xTh = sb1.tile([128, KD, BC], dt.bfloat16)
            nc.sync.dma_start(
                xTh[:], xTh_d.ap().rearrange("(po pi) b -> pi po b", pi=128))
            xTl = sb1.tile([128, KD, BC], dt.bfloat16)
            nc.sync.dma_start(
                xTl[:], xTl_d.ap().rearrange("(po pi) b -> pi po b", pi=128))
        if has_benc:
            beff = sb1.tile([1, F], dt.float32)
            nc.sync.dma_start(beff[:], beff_d.ap())
        if has_bdec:
            bdec1 = sb1.tile([1, D], dt.float32)
            nc.sync.dma_start(bdec1[:], bdec_d.ap())
            bdec_bc = sb1.tile([128, D], dt.float32)
            nc.gpsimd.partition_broadcast(bdec_bc[:], bdec1[:])

        v256s = [sb1.tile([128, NC8], dt.float32, name=f"v256_{bt}")
                 for bt in range(NBT)]
        i256s = [sb1.tile([128, NC8], dt.uint16, name=f"i256_{bt}")
                 for bt in range(NBT)]

        fboff = sb1.tile([128, NC8], dt.uint16)
        nc.gpsimd.iota(fboff[:], pattern=[[512, NFB], [0, 8]], base=0,
                       channel_multiplier=0)
        piota = sb1.tile([128, NC8], dt.uint16)
        nc.gpsimd.iota(piota[:], pattern=[[1, NC8]], base=0,
                       channel_multiplier=0)

        # identity row-mask: identv[p, j] = s * (j == p), s = decode descale
        dscale = 2.0 ** -15 if ENC_MODE == "fp16fp8" else 1.0
        if DVE_DECODE:
            gall = sb1.tile([128, K, D], dt.float16)
        else:
            ones1 = sb1.tile([128, 1], dt.float32)
            nc.vector.memset(ones1[:], dscale)
            identv = sb1.tile([128, 128], dt.float16)
            nc.gpsimd.affine_select(
                out=identv[:], in_=ones1[:, :1].to_broadcast([128, 128]),
                pattern=[[1, 128]], compare_op=Alu.is_equal, fill=0.0,
                base=0, channel_multiplier=-1)

        wh_v = wencTh_d.ap().rearrange("(po pi) f -> pi po f", pi=128)
        if ENC_MODE == "fp16fp8":
            wc_v = wc8_d.ap().rearrange("(kp i pi) f -> pi kp i f",
                                        pi=128, i=2)
        else:
            wl_v = wencTl_d.ap().rearrange("(po pi) f -> pi po f", pi=128)

        def encode_blocks(bts):
          for fb in range(NFB):
            fsl = slice(fb * 512, (fb + 1) * 512)
            if ENC_MODE == "fp16fp8":
                wbh = sb.tile([128, KD, 512], dt.float16, tag="wbh")
                nc.sync.dma_start(wbh[:], wh_v[:, :, fsl])
                wbc = sb.tile([128, KD, 2, 512], dt.float8e4, tag="wbc")
                nc.sync.dma_start(wbc[:], wc_v[:, :, :, fsl])
            else:
                wbh = sb.tile([128, KD, 512], dt.bfloat16, tag="wbh")
                nc.sync.dma_start(wbh[:], wh_v[:, :, fsl])
                wbl = sb.tile([128, KD, 512], dt.bfloat16, tag="wbl")
                nc.sync.dma_start(wbl[:], wl_v[:, :, fsl])
            if has_benc:
                beff_bc = stp.tile([128, 512], dt.float32, tag="beffbc")
                nc.gpsimd.partition_broadcast(
                    beff_bc[:], beff[:, fb * 512:(fb + 1) * 512])
            for bt in bts:
                bsl = slice(bt * 128, (bt + 1) * 128)
                if ENC_MODE == "fp16fp8":
                    # main operands are pre-scaled 2^8/2^7 on the host so the
                    # fp16 main and fp8 corrections share the 2^15 scale and
                    # one PSUM accumulation group; 2^-15 folds into `identv`.
                    ps = psA.tile([128, 512], dt.float32, tag="encps")
                    for k in range(KD):
                        nc.tensor.matmul(ps[:], lhsT=xTh[:, k, bsl],
                                         rhs=wbh[:, k, :],
                                         start=(k == 0), stop=False)
                    if FP8_PLAIN:
                        for k in range(KD):
                            for i in range(2):
                                nc.tensor.matmul(
                                    ps[:], lhsT=xc8[:, k, i, bsl],
                                    rhs=wbc[:, k, i, :],
                                    start=False,
                                    stop=(k == KD - 1 and i == 1))
                    else:
                        for k in range(KD):
                            nc.tensor.matmul(
                                ps[:], lhsT=xc8[:, k, :, bsl],
                                rhs=wbc[:, k, :, :],
                                perf_mode=mybir.MatmulPerfMode.DoubleRow,
                                start=False, stop=(k == KD - 1))
                    if has_benc:
                        nc.vector.tensor_add(ps[:], ps[:], beff_bc[:])
                    src = ps
                else:
                    ps = psA.tile([128, 512], dt.float32, tag="encps")
                    terms = [(xTh, wbh), (xTh, wbl), (xTl, wbh)]
                    n_mm = KD * len(terms)
                    i = 0
                    for k in range(KD):
                        for (a, w_) in terms:
                            nc.tensor.matmul(ps[:], lhsT=a[:, k, bsl],
                                             rhs=w_[:, k, :],
                                             start=(i == 0),
                                             stop=(i == n_mm - 1))
                            i += 1
                    if has_benc:
                        nc.vector.tensor_add(ps[:], ps[:], beff_bc[:])
                    src = ps
                nc.vector.max(out=v256s[bt][:, fb * 8:(fb + 1) * 8],
                              in_=src[:])
                nc.vector.max_index(out=i256s[bt][:, fb * 8:(fb + 1) * 8],
                                    in_max=v256s[bt][:, fb * 8:(fb + 1) * 8],
                                    in_values=src[:])

        def tail(bt):
            # candidate local idx -> global feature idx
            gidx = sb.tile([128, NC8], dt.uint16, tag="gidx")
            nc.vector.tensor_tensor(gidx[:], i256s[bt][:], fboff[:], op=Alu.add)
            if DEBUG_TOP8 and bt == 0:
                nc.sync.dma_start(dbg_v.ap(), v256s[bt][:])
                nc.sync.dma_start(dbg_i.ap(), i256s[bt][:])
                nc.sync.dma_start(dbg_gidx.ap(), gidx[:])

            # exact top-32 of the 256 block-top-8 candidates (in-place)
            cv = v256s[bt][:]
            vals = sb.tile([128, K], dt.float32, tag="vals")
            tpos = sb.tile([128, K], dt.uint16, tag="tpos")
            for r in range(NCH):
                m8 = vals[:, r * 8:(r + 1) * 8]
                nc.vector.max(out=m8, in_=cv)
                nc.vector.max_index(out=tpos[:, r * 8:(r + 1) * 8],
                                    in_max=m8, in_values=cv)
                if r != NCH - 1:
                    nc.vector.match_replace(out=cv, in_to_replace=m8,
                                            in_values=cv, imm_value=-1e30)

            # feature idx lookup: one-hot(pos) * gidx, reduce-max
            eq = sb.tile([128, K, NC8], dt.uint16, tag="eq")
            nc.vector.tensor_tensor(
                eq[:], tpos[:, :, None].to_broadcast([128, K, NC8]),
                piota[:, None, :].to_broadcast([128, K, NC8]), op=Alu.is_equal)
            nc.vector.tensor_tensor(
                eq[:], eq[:], gidx[:, None, :].to_broadcast([128, K, NC8]),
                op=Alu.mult)
            idx16 = sb.tile([128, K], dt.uint16, tag="idx16")
            nc.vector.tensor_reduce(idx16[:], eq[:],
                                    axis=mybir.AxisListType.X, op=Alu.max)
            idx32 = sb.tile([128, K], dt.uint32, tag="idx32")
            nc.vector.tensor_copy(idx32[:], idx16[:])

            if DEBUG_TOP8 and bt == 0:
                nc.sync.dma_start(dbg_tpos.ap(), tpos[:])
                nc.sync.dma_start(dbg_vals.ap(), vals[:])
                nc.sync.dma_start(dbg_idx.ap(), idx32[:])

            # relu guard (rows with <32 positive preacts decode the extras as 0)
            nc.vector.tensor_scalar_max(vals[:], vals[:], 0.0)

            if DVE_DECODE:
                # xhat row p = sum_c vals[p,c] * wdec[idx[p,c], :] on the DVE:
                # scale vals once (descale), gather all 32 rows, multiply by
                # the broadcast coefficients and reduce over c (strided view).
                if dscale != 1.0:
                    nc.vector.tensor_scalar(vals[:], vals[:], dscale, None,
                                            op0=Alu.mult)
                for c in range(K):
                    nc.gpsimd.indirect_dma_start(
                        out=gall[:, c], out_offset=None,
                        in_=wdecT_d.ap(),
                        in_offset=bass.IndirectOffsetOnAxis(
                            ap=idx32[:, c:c + 1], axis=0))
                nc.vector.tensor_tensor(
                    gall[:], gall[:],
                    vals[:, :, None].to_broadcast([128, K, D]), op=Alu.mult)
                osb = stp.tile([128, D], dt.float32, tag="osb")
                nc.vector.tensor_reduce(
                    osb[:], gall[:].rearrange("p c d -> p d c"),
                    axis=mybir.AxisListType.X, op=Alu.add)
                if has_bdec:
                    nc.vector.tensor_add(osb[:], osb[:], bdec_bc[:])
            else:
                # diag(vals) via identity-mask multiply (identv holds the
                # 2^-15 descale in fp16fp8 mode)
                diag = sb.tile([128, K, 128], dt.float16, tag="diag")
                nc.vector.tensor_tensor(
                    diag[:], vals[:, :, None].to_broadcast([128, K, 128]),
                    identv[:, None, :].to_broadcast([128, K, 128]),
                    op=Alu.mult)

                pso = psB.tile([128, D], dt.float32, tag="decps")
                for c in range(K):
                    g = sb.tile([128, D], dt.float16, tag="g")
                    nc.gpsimd.indirect_dma_start(
                        out=g[:], out_offset=None,
                        in_=wdecT_d.ap(),
                        in_offset=bass.IndirectOffsetOnAxis(
                            ap=idx32[:, c:c + 1], axis=0))
                    nc.tensor.matmul(pso[:, :512], lhsT=diag[:, c, :],
                                     rhs=g[:, :512],
                                     start=(c == 0), stop=(c == K - 1))
                    nc.tensor.matmul(pso[:, 512:D], lhsT=diag[:, c, :],
                                     rhs=g[:, 512:D],
                                     start=(c == 0), stop=(c == K - 1))
                osb = stp.tile([128, D], dt.float32, tag="osb")
                if has_bdec:
                    nc.vector.tensor_add(osb[:], pso[:], bdec_bc[:])
                else:
                    nc.scalar.copy(osb[:], pso[:])
            nc.sync.dma_start(out_d.ap()[bt * 128:(bt + 1) * 128, :], osb[:])

        # uneven split: earlier groups larger, last group smallest so the
        # only non-overlapped tail (the final group's) is as short as possible
        base, rem = divmod(NBT, HALVES)
        sizes = [base + (1 if i < rem else 0) for i in range(HALVES)]
        groups, s = [], 0
        for sz in sizes:
            groups.append(list(range(s, s + sz)))
            s += sz
        for bts in groups:
            encode_blocks(bts)
            for bt in bts:
                tail(bt)

    from contextlib import ExitStack
    with tile.TileContext(nc) as tc:
        with ExitStack() as ctx:
            pools = (
                ctx.enter_context(tc.tile_pool(name="sb", bufs=2)),
                ctx.enter_context(tc.tile_pool(name="sb1", bufs=1)),
                ctx.enter_context(tc.tile_pool(name="stage", bufs=4)),
                ctx.enter_context(tc.tile_pool(name="psA", bufs=4,
                                               space="PSUM")),
                ctx.enter_context(tc.tile_pool(name="psB", bufs=2, space="PSUM")),
                ctx.enter_context(tc.tile_pool(name="psC", bufs=1, space="PSUM")),
            )
            if repeat == 1:
                body(tc, pools)
            else:
                with tc.For_i(0, repeat, 1):
                    body(tc, pools)

    if FIX_SYNC:
        _fix_sync_waits(nc)
    return nc


def _build(has_benc: bool, has_bdec: bool, repeat: int = 1):
    if DESIGN == "top8":
        return _build_top8(has_benc, has_bdec, repeat)
    import concourse.bass as bass
    import concourse.mybir as mybir
    import concourse.tile as tile
    dt = mybir.dt
    Alu = mybir.AluOpType

    nc = bass.Bass("TRN2", target_bir_lowering=False, debug=False,
                   num_devices=N_CORES)

    if ENC_MODE == "bf16x3":
        xTh_d = nc.dram_tensor("xTh", [D, BC], dt.bfloat16, kind="ExternalInput")
        xTl_d = nc.dram_tensor("xTl", [D, BC], dt.bfloat16, kind="ExternalInput")
        wencTh_d = nc.dram_tensor("wencTh", [D, F], dt.bfloat16,
                                  kind="ExternalInput")
        wencTl_d = nc.dram_tensor("wencTl", [D, F], dt.bfloat16,
                                  kind="ExternalInput")
    else:
        enc_dt = dt.float32r if ENC_MODE == "f32r" else dt.float32
        xT_d = nc.dram_tensor("xT", [D, BC], enc_dt, kind="ExternalInput")
        wencT_d = nc.dram_tensor("wencT", [D, F], enc_dt,
                                 kind="ExternalInput")
    wdecT_d = nc.dram_tensor("wdecT16", [F, D], dt.float16, kind="ExternalInput")
    beff_d = nc.dram_tensor("beff", [1, F], dt.float32, kind="ExternalInput")
    bdec_d = nc.dram_tensor("bdec", [1, D], dt.float32, kind="ExternalInput")
    out_d = nc.dram_tensor("xhat", [BC, D], dt.float32, kind="ExternalOutput")
    preact_d = nc.dram_tensor("preact_spill", [BC, F], dt.float32)

    preact_flat = preact_d.ap().rearrange("b (c w) -> (b c) w", w=CH)

    def body(tc, pools):
        sb, sb1, stp, psA, psB = pools

        # resident inputs
        if ENC_MODE == "bf16x3":
            xTh = sb1.tile([128, KD, BC], dt.bfloat16)
            nc.sync.dma_start(
                xTh[:], xTh_d.ap().rearrange("(po pi) b -> pi po b", pi=128))
            xTl = sb1.tile([128, KD, BC], dt.bfloat16)
            nc.sync.dma_start(
                xTl[:], xTl_d.ap().rearrange("(po pi) b -> pi po b", pi=128))
        else:
            xT = sb1.tile([128, KD, BC], enc_dt)
            nc.sync.dma_start(
                xT[:], xT_d.ap().rearrange("(po pi) b -> pi po b", pi=128))
        if has_benc:
            beff = sb1.tile([1, F], dt.float32)
            nc.sync.dma_start(beff[:], beff_d.ap())
        if has_bdec:
            bdec1 = sb1.tile([1, D], dt.float32)
            nc.sync.dma_start(bdec1[:], bdec_d.ap())
            bdec_bc = sb1.tile([128, D], dt.float32)
            nc.gpsimd.partition_broadcast(bdec_bc[:], bdec1[:])

        cms = [sb1.tile([128, C], dt.float32, name=f"cm{bt}") for bt in range(NBT)]

        # ---------------- encode + spill + chunk-max ----------------
        if ENC_MODE == "bf16x3":
            wh_v = wencTh_d.ap().rearrange("(po pi) f -> pi po f", pi=128)
            wl_v = wencTl_d.ap().rearrange("(po pi) f -> pi po f", pi=128)
        else:
            wencT_v = wencT_d.ap().rearrange("(po pi) f -> pi po f", pi=128)
        def encode_blocks(bts):
          for fb in range(NFB):
            if ENC_MODE == "bf16x3":
                wbh = sb.tile([128, KD, 512], dt.bfloat16, tag="wbh")
                nc.sync.dma_start(wbh[:], wh_v[:, :, fb * 512:(fb + 1) * 512])
                wbl = sb.tile([128, KD, 512], dt.bfloat16, tag="wbl")
                nc.sync.dma_start(wbl[:], wl_v[:, :, fb * 512:(fb + 1) * 512])
            else:
                wb = sb.tile([128, KD, 512], enc_dt, tag="wb")
                nc.sync.dma_start(wb[:], wencT_v[:, :, fb * 512:(fb + 1) * 512])
            if has_benc:
                beff_bc = stp.tile([128, 512], dt.float32, tag="beffbc")
                nc.gpsimd.partition_broadcast(
                    beff_bc[:], beff[:, fb * 512:(fb + 1) * 512])
            for bt in bts:
                ps = psA.tile([128, 512], dt.float32, tag="encps")
                if ENC_MODE == "bf16x3":
                    bsl = slice(bt * 128, (bt + 1) * 128)
                    terms = [(xTh, wbh), (xTh, wbl), (xTl, wbh)]
                    n_mm = KD * len(terms)
                    i = 0
                    for k in range(KD):
                        for (a, w_) in terms:
                            nc.tensor.matmul(ps[:], lhsT=a[:, k, bsl],
                                             rhs=w_[:, k, :],
                                             start=(i == 0), stop=(i == n_mm - 1))
                            i += 1
                else:
                    for k in range(KD):
                        lhsT = xT[:, k, bt * 128:(bt + 1) * 128]
                        rhs = wb[:, k, :]
                        nc.tensor.matmul(ps[:], lhsT=lhsT, rhs=rhs,
                                         start=(k == 0), stop=(k == KD - 1))
                stage = stp.tile([128, 512], dt.float32, tag="stage")
                if has_benc:
                    nc.vector.tensor_add(stage[:], ps[:], beff_bc[:])
                    nc.vector.tensor_reduce(
                        cms[bt][:, fb * 8:(fb + 1) * 8],
                        stage[:].rearrange("p (c w) -> p c w", w=CH),
                        axis=mybir.AxisListType.X, op=Alu.max)
                else:
                    nc.scalar.copy(stage[:], ps[:])
                    nc.vector.tensor_reduce(
                        cms[bt][:, fb * 8:(fb + 1) * 8],
                        ps[:].rearrange("p (c w) -> p c w", w=CH),
                        axis=mybir.AxisListType.X, op=Alu.max)
                nc.sync.dma_start(
                    preact_d.ap()[bt * 128:(bt + 1) * 128,
                                  fb * 512:(fb + 1) * 512], stage[:])

        # iota constants for topk/index math
        jiota = sb1.tile([128, K], dt.uint32)
        nc.gpsimd.iota(jiota[:], pattern=[[1, K]], base=0, channel_multiplier=0)

        # ---------------- per-b-tile topk + compact decode ----------------
        def tail(bt):
            # top-32 chunks by chunk max
            cmw = sb.tile([128, C], dt.float32, tag="cmw")
            nc.vector.tensor_copy(cmw[:], cms[bt][:])
            cm8 = sb.tile([128, 8], dt.float32, tag="cm8")
            chunkid = sb.tile([128, K], dt.uint32, tag="chunkid")
            for r in range(NCH):
                nc.vector.max(out=cm8[:], in_=cmw[:])
                nc.vector.max_index(out=chunkid[:, r * 8:(r + 1) * 8],
                                    in_max=cm8[:], in_values=cmw[:])
                if r != NCH - 1:
                    nc.vector.match_replace(out=cmw[:], in_to_replace=cm8[:],
                                            in_values=cmw[:], imm_value=-1e30)

            # gather the 32 chunks' contents from the DRAM spill
            rowoff = sb.tile([128, 1], dt.uint32, tag="rowoff")
            nc.gpsimd.iota(rowoff[:], pattern=[[1, 1]], base=bt * 128 * C,
                           channel_multiplier=C)
            off = sb.tile([128, K], dt.uint32, tag="off")
            nc.vector.tensor_tensor(off[:], chunkid[:],
                                    rowoff[:, :1].to_broadcast([128, K]),
                                    op=Alu.add)
            cand = sb.tile([128, K, CH], dt.float32, tag="cand")
            if BATCH_GATHER:
                nc.gpsimd.indirect_dma_start(
                    out=cand[:], out_offset=None,
                    in_=preact_flat,
                    in_offset=bass.IndirectOffsetOnAxis(ap=off[:], axis=0))
            else:
                for j in range(K):
                    nc.gpsimd.indirect_dma_start(
                        out=cand[:, j], out_offset=None,
                        in_=preact_flat,
                        in_offset=bass.IndirectOffsetOnAxis(ap=off[:, j:j + 1],
                                                            axis=0))

            # exact top-32 of the candidates
            candf = cand[:].rearrange("p a b -> p (a b)")
            vals = sb.tile([128, K], dt.float32, tag="vals")
            pos = sb.tile([128, K], dt.uint32, tag="pos")
            for r in range(NCH):
                m8 = vals[:, r * 8:(r + 1) * 8]
                nc.vector.max(out=m8, in_=candf)
                nc.vector.max_index(out=pos[:, r * 8:(r + 1) * 8],
                                    in_max=m8, in_values=candf)
                if r != NCH - 1:
                    nc.vector.match_replace(out=candf, in_to_replace=m8,
                                            in_values=candf, imm_value=-1e30)

            # positions -> global feature indices:
            # idx = chunkid[p, pos>>6]*64 + (pos&63), chunkid lookup done as a
            # one-hot compare-multiply-reduce (no per-partition gather on HW)
            j32 = sb.tile([128, K], dt.uint32, tag="j32")
            nc.vector.tensor_scalar(j32[:], pos[:], 6, None,
                                    op0=Alu.logical_shift_right)
            l32 = sb.tile([128, K], dt.uint32, tag="l32")
            nc.vector.tensor_scalar(l32[:], pos[:], CH - 1, None,
                                    op0=Alu.bitwise_and)
            eq = sb.tile([128, K, K], dt.uint32, tag="eq")
            nc.vector.tensor_tensor(
                eq[:], j32[:, :, None].to_broadcast([128, K, K]),
                jiota[:, None, :].to_broadcast([128, K, K]), op=Alu.is_equal)
            nc.vector.tensor_tensor(
                eq[:], eq[:], chunkid[:, None, :].to_broadcast([128, K, K]),
                op=Alu.mult)
            cs32 = sb.tile([128, K], dt.uint32, tag="cs32")
            nc.vector.tensor_reduce(cs32[:], eq[:],
                                    axis=mybir.AxisListType.X, op=Alu.max)
            idx32 = sb.tile([128, K], dt.uint32, tag="idx32")
            nc.vector.tensor_scalar(idx32[:], cs32[:], 6, None,
                                    op0=Alu.logical_shift_left)
            nc.vector.tensor_tensor(idx32[:], idx32[:], l32[:], op=Alu.add)

            # relu guard (if a row has <32 positive preacts, the reference's
            # extra top-k entries are relu zeros; zero coefficients match it)
            nc.vector.tensor_scalar_max(vals[:], vals[:], 0.0)

            # diag(vals[:, c]) tiles, fp16
            diag = sb.tile([128, K, 128], dt.float16, tag="diag")
            nc.gpsimd.affine_select(
                out=diag[:],
                in_=vals[:, :, None].to_broadcast([128, K, 128]),
                pattern=[[0, K], [1, 128]],
                compare_op=Alu.is_equal, fill=0.0,
                base=0, channel_multiplier=-1)

            # compact decode: xhat_tile = sum_c diag(vals[:,c]) @ WdecT[idx[:,c]]
            pso = psB.tile([128, D], dt.float32, tag="decps")
            if BATCH_GATHER:
                GH = K // 2            # gather W_dec rows in 2 half-batches
                for h in range(2):
                    g = sb.tile([128, GH, D], dt.float16, tag="g")
                    nc.gpsimd.indirect_dma_start(
                        out=g[:], out_offset=None,
                        in_=wdecT_d.ap(),
                        in_offset=bass.IndirectOffsetOnAxis(
                            ap=idx32[:, h * GH:(h + 1) * GH], axis=0))
                    for ci in range(GH):
                        c = h * GH + ci
                        nc.tensor.matmul(pso[:, :512], lhsT=diag[:, c, :],
                                         rhs=g[:, ci, :512],
                                         start=(c == 0), stop=(c == K - 1))
                        nc.tensor.matmul(pso[:, 512:D], lhsT=diag[:, c, :],
                                         rhs=g[:, ci, 512:D],
                                         start=(c == 0), stop=(c == K - 1))
            else:
                for c in range(K):
                    g = sb.tile([128, D], dt.float16, tag="g")
                    nc.gpsimd.indirect_dma_start(
                        out=g[:], out_offset=None,
                        in_=wdecT_d.ap(),
                        in_offset=bass.IndirectOffsetOnAxis(ap=idx32[:, c:c + 1],
                                                            axis=0))
                    nc.tensor.matmul(pso[:, :512], lhsT=diag[:, c, :],
                                     rhs=g[:, :512],
                                     start=(c == 0), stop=(c == K - 1))
                    nc.tensor.matmul(pso[:, 512:D], lhsT=diag[:, c, :],
                                     rhs=g[:, 512:D],
                                     start=(c == 0), stop=(c == K - 1))
            osb = stp.tile([128, D], dt.float32, tag="osb")
            if has_bdec:
                nc.vector.tensor_add(osb[:], pso[:], bdec_bc[:])
            else:
                nc.scalar.copy(osb[:], pso[:])
            nc.sync.dma_start(out_d.ap()[bt * 128:(bt + 1) * 128, :], osb[:])

        if HALVES == 1:
            groups = [list(range(NBT))]
        else:
            h = NBT // HALVES
            groups = [list(range(i * h, (i + 1) * h)) for i in range(HALVES)]
        for bts in groups:
            encode_blocks(bts)
            for bt in bts:
                tail(bt)

    from contextlib import ExitStack
    with tile.TileContext(nc) as tc:
        with ExitStack() as ctx:
            pools = (
                ctx.enter_context(tc.tile_pool(name="sb", bufs=2)),
                ctx.enter_context(tc.tile_pool(name="sb1", bufs=1)),
                ctx.enter_context(tc.tile_pool(name="stage", bufs=4)),
                ctx.enter_context(tc.tile_pool(name="psA", bufs=4, space="PSUM")),
                ctx.enter_context(tc.tile_pool(name="psB", bufs=2, space="PSUM")),
            )
            if repeat == 1:
                body(tc, pools)
            else:
                with tc.For_i(0, repeat, 1):
                    body(tc, pools)

    if FIX_SYNC:
        _fix_sync_waits(nc)
    return nc


def _patch_ldw_opt():
    """Rewrite the hardcoded --enable-ldw-opt=false walrus flag at runtime."""
    from concourse import bass_utils
    if getattr(bass_utils, "_ldw_patched", False):
        return
    orig = bass_utils.run_command

    def run_command(cmd, *a, **k):
        if LDW_OPT and isinstance(cmd, list):
            cmd = ["--enable-ldw-opt=true" if c == "--enable-ldw-opt=false"
                   else c for c in cmd]
        return orig(cmd, *a, **k)

    bass_utils.run_command = run_command
    bass_utils._ldw_patched = True


def _get_runner(has_benc, has_bdec, repeat=1):
    key = (has_benc, has_bdec, repeat, ENC_MODE, HALVES, BATCH_GATHER, DESIGN,
           FP8_PLAIN, DVE_DECODE, LDW_OPT)
    if key in _cache:
        return _cache[key]
    import jax
    from jax.sharding import Mesh, PartitionSpec
    from jax.experimental.shard_map import shard_map
    import concourse.mybir as mybir
    from concourse import bass2jax
    from concourse.bass2jax import _bass_exec_p, install_neuronx_cc_hook

    nc = _build(has_benc, has_bdec, repeat)
    _patch_ldw_opt()
    install_neuronx_cc_hook()

    partition_name = (nc.partition_id_tensor.name
                      if nc.partition_id_tensor else None)
    in_names, out_names, out_avals, zero_outs = [], [], [], []
    for alloc in nc.m.functions[0].allocations:
        if not isinstance(alloc, mybir.MemoryLocationSet):
            continue
        name = alloc.memorylocations[0].name
        if alloc.kind == "ExternalInput":
            if name != partition_name:
                in_names.append(name)
        elif alloc.kind == "ExternalOutput":
            shape = tuple(alloc.tensor_shape)
            dtype = mybir.dt.np(alloc.dtype)
            out_names.append(name)
            out_avals.append(jax.core.ShapedArray(shape, dtype))
            zero_outs.append(np.zeros(shape, dtype))
    n_params = len(in_names)
    all_in = in_names + out_names
    if partition_name is not None:
        all_in = all_in + [partition_name]

    def _bodyfn(*args):
        operands = list(args)
        if partition_name is not None:
            operands.append(bass2jax.partition_id_tensor())
        outs = _bass_exec_p.bind(
            *operands, out_avals=tuple(out_avals), in_names=tuple(all_in),
            out_names=tuple(out_names), lowering_input_output_aliases=(),
            sim_require_finite=True, sim_require_nnan=True, nc=nc)
        return tuple(outs)

    try:
        devices = jax.devices("axon")[:N_CORES]
    except Exception:
        devices = jax.devices()[:N_CORES]
    mesh = Mesh(np.asarray(devices), ("core",))
    n_outs = len(out_names)
    fn = jax.jit(
        shard_map(_bodyfn, mesh=mesh,
                  in_specs=(PartitionSpec("core"),) * (n_params + n_outs),
                  out_specs=(PartitionSpec("core"),) * n_outs,
                  check_rep=False),
        keep_unused=True)
    sharding = jax.sharding.NamedSharding(mesh, PartitionSpec("core"))
    r = {"fn": fn, "in_names": in_names, "out_names": out_names,
         "zero_outs": zero_outs, "nc": nc, "sharding": sharding}
    _cache[key] = r
    return r


def _prep_host(x, W_enc, b_enc, W_dec, b_dec):
    x_eff = x - b_dec[None, :]
    xT_full = np.ascontiguousarray(x_eff.T, dtype=np.float32)      # [D, B]
    wencT = np.ascontiguousarray(W_enc.T, dtype=np.float32)        # [D, F]
    wdecT16 = np.ascontiguousarray(W_dec.T, dtype=np.float16)      # [F, D]
    beff = (b_enc.astype(np.float64)
            - W_enc.astype(np.float64) @ b_dec.astype(np.float64))
    beff = beff.astype(np.float32)[None, :]                        # [1, F]
    bdec = b_dec.astype(np.float32)[None, :]                       # [1, D]
    return xT_full, wencT, wdecT16, beff, bdec


def kernel(x, W_enc, b_enc, W_dec, b_dec, _repeat=1, _timeit=False):
    x = np.asarray(x, np.float32)
    W_enc = np.asarray(W_enc, np.float32)
    b_enc = np.asarray(b_enc, np.float32)
    W_dec = np.asarray(W_dec, np.float32)
    b_dec = np.asarray(b_dec, np.float32)
    xT_full, wencT, wdecT16, beff, bdec = _prep_host(x, W_enc, b_enc, W_dec, b_dec)
    has_benc = bool(np.any(beff))
    has_bdec = bool(np.any(b_dec))
    r = _get_runner(has_benc, has_bdec, _repeat)

    per_core = {
        "wdecT16": [wdecT16] * N_CORES,
        "beff": [beff] * N_CORES,
        "bdec": [bdec] * N_CORES,
    }
    if ENC_MODE == "fp16fp8":
        import ml_dtypes
        f8 = ml_dtypes.float8_e4m3
        Bfull = xT_full.shape[1]
        # main operands pre-scaled 2^8 / 2^7 so main & correction PSUM terms
        # share the 2^15 scale (descale folds into the decode identity mask)
        xh16 = (xT_full * 2.0 ** 8).astype(np.float16)
        xr = xT_full - xh16.astype(np.float32) * 2.0 ** -8
        xc8 = np.empty([KD, 2, 128, Bfull], f8)
        wc8 = np.empty([KD, 2, 128, F], f8)
        for k in range(KD):
            rs = slice(k * 128, (k + 1) * 128)
            xc8[k, 0] = (xr[rs] * 2.0 ** 11).astype(f8)
            xc8[k, 1] = xT_full[rs].astype(f8)
        wh16 = (wencT * 2.0 ** 7).astype(np.float16)
        wr = wencT - wh16.astype(np.float32) * 2.0 ** -7
        for k in range(KD):
            rs = slice(k * 128, (k + 1) * 128)
            wc8[k, 0] = (wencT[rs] * 2.0 ** 4).astype(f8)
            wc8[k, 1] = (wr[rs] * 2.0 ** 15).astype(f8)
        xc8 = xc8.reshape(2 * D, Bfull)
        wc8 = wc8.reshape(2 * D, F)
        beff = beff * 2.0 ** 15
        per_core["xTh"] = [np.ascontiguousarray(xh16[:, c * BC:(c + 1) * BC])
                           for c in range(N_CORES)]
        per_core["xc8"] = [np.ascontiguousarray(xc8[:, c * BC:(c + 1) * BC])
                           for c in range(N_CORES)]
        per_core["wencTh"] = [wh16] * N_CORES
        per_core["wc8"] = [wc8] * N_CORES
    elif ENC_MODE == "bf16x3":
        import ml_dtypes
        bf16 = ml_dtypes.bfloat16
        xTh = xT_full.astype(bf16)
        xTl = (xT_full - xTh.astype(np.float32)).astype(bf16)
        wh = wencT.astype(bf16)
        wl = (wencT - wh.astype(np.float32)).astype(bf16)
        per_core["xTh"] = [np.ascontiguousarray(xTh[:, c * BC:(c + 1) * BC])
                           for c in range(N_CORES)]
        per_core["xTl"] = [np.ascontiguousarray(xTl[:, c * BC:(c + 1) * BC])
                           for c in range(N_CORES)]
        per_core["wencTh"] = [wh] * N_CORES
        per_core["wencTl"] = [wl] * N_CORES
    else:
        per_core["xT"] = [np.ascontiguousarray(xT_full[:, c * BC:(c + 1) * BC])
                          for c in range(N_CORES)]
        per_core["wencT"] = [wencT] * N_CORES
    args = [np.concatenate(per_core[name], axis=0) for name in r["in_names"]]
    args += [np.concatenate([z] * N_CORES, axis=0) for z in r["zero_outs"]]

    import jax, time
    dev_args = [jax.device_put(a, r["sharding"]) for a in args]
    kernel.last_dev_args = dev_args
    kernel.last_runner = r
    outs = r["fn"](*dev_args)
    jax.block_until_ready(outs)
    if _timeit:
        times = []
        for _ in range(_timeit if isinstance(_timeit, int) and _timeit > 1 else 8):
            t0 = time.perf_counter()
            outs = r["fn"](*dev_args)
            jax.block_until_ready(outs)
            times.append(time.perf_counter() - t0)
        kernel.last_times = times

    xhat = np.asarray(outs[r["out_names"].index("xhat")])  # [B, D] concat
    return xhat.astype(np.float32)

